# revision 7
# baseline (speedup 1.0000x reference)
"""Sparse-attention wrapper kernel for 8 trn2 NeuronCores.

Sharding: core c -> (b = c // 4, g = c % 4). Data-parallel over batch B=2,
tensor-parallel over the 4 KV head groups (4 q-heads / 1 kv-head each).
Per-core pipeline (all matmuls fp32r on the PE):
  A) streamed projections: kT/vT = Wk/Wv^T @ hiddenT, qT = Wq^T @ hidden_qT,
     with RMS statistics (ln-norm + q/k-norm) fused as column scales and
     RoPE applied via host-precomputed cos/sin factor tensors.
  B) per-head attention in transposed orientation: scoresT[s,k] tiles,
     exp on ACT, causal handling via host-derived column ranges + boundary
     masks, unnormalized attn@v and row-sums via ones-matmul, late 1/rowsum
     normalization.
  C) o_proj into oT[D,K] partials (+ bo/4), on-device ReduceScatter over the
     4 cores of each batch, host scatters the K rows back into [B,S,D] zeros.
"""

import numpy as np
import concourse.bacc as bacc
import concourse.tile as tile
from concourse import mybir
from concourse.bass_utils import run_bass_kernel_spmd

B, S, K, D, H, HKV, HD = 2, 2048, 1024, 2048, 16, 4, 128
EPS = 1e-6
SCALE = float(HD) ** -0.5
NCORES = 8
NT = S // 128          # 16 s-tiles
NDC = D // 128         # 16 d-chunks
QH = H // HKV          # 4 q-heads per core
GW = QH * HD           # 512 columns of Wq per core

F32 = mybir.dt.float32
F32R = mybir.dt.float32r

_BUILD_CACHE = {}


def _chunks(lo, hi, bank=512):
    """Split [lo, hi) at multiples of `bank` (PSUM bank boundaries)."""
    out = []
    a = lo
    while a < hi:
        b = min(hi, (a // bank + 1) * bank)
        out.append((a, b))
        a = b
    return out


def _build(klo_u, khi_max):
    """Build the SPMD kernel. klo_u[t]: first computed column of s-tile t;
    khi_max[t]: end of the masked boundary region of s-tile t."""
    nc = bacc.Bacc("TRN2", target_bir_lowering=False, debug=False,
                   num_devices=NCORES)

    mw = [max(0, khi_max[t] - klo_u[t]) for t in range(NT)]
    moff = np.concatenate([[0], np.cumsum(mw)]).astype(int)
    MW = int(moff[-1])

    hT = nc.declare_dram_parameter("hT", [D, S], F32R, isOutput=False)
    hqT = nc.declare_dram_parameter("hqT", [D, K], F32R, isOutput=False)
    wq = nc.declare_dram_parameter("wq", [D, GW], F32R, isOutput=False)
    wk = nc.declare_dram_parameter("wk", [D, HD], F32R, isOutput=False)
    wv = nc.declare_dram_parameter("wv", [D, HD], F32R, isOutput=False)
    wo = nc.declare_dram_parameter("wo", [GW, D], F32R, isOutput=False)
    cosq = nc.declare_dram_parameter("cosq", [HD, K], F32, isOutput=False)
    sinq = nc.declare_dram_parameter("sinq", [HD, K], F32, isOutput=False)
    cosk = nc.declare_dram_parameter("cosk", [HD, S], F32, isOutput=False)
    sink = nc.declare_dram_parameter("sink", [HD, S], F32, isOutput=False)
    maskp = nc.declare_dram_parameter("maskp", [128, max(MW, 1)], F32,
                                      isOutput=False)
    bo4 = nc.declare_dram_parameter("bo4", [128, NDC], F32, isOutput=False)
    onesp = nc.declare_dram_parameter("onesp", [128, 1], F32R, isOutput=False)
    epsp = nc.declare_dram_parameter("epsp", [1, 1], F32, isOutput=False)
    ident = nc.declare_dram_parameter("ident", [128, 128], F32R, isOutput=False)
    oshard = nc.declare_dram_parameter("oshard", [D // 4, K], F32,
                                       isOutput=True)

    with tile.TileContext(nc) as tc:
        _emit(nc, tc, locals(), klo_u, khi_max, moff)
    nc.finalize()
    return nc


def _emit(nc, tc, io, klo_u, khi_max, moff):
    hT, hqT = io["hT"], io["hqT"]
    wq, wk, wv, wo = io["wq"], io["wk"], io["wv"], io["wo"]
    cosq, sinq, cosk, sink = io["cosq"], io["sinq"], io["cosk"], io["sink"]
    maskp, bo4, onesp, ident = io["maskp"], io["bo4"], io["onesp"], io["ident"]
    epsp = io["epsp"]
    oshard = io["oshard"]

    ctx_pools = []

    def pool(name, bufs=1, space="SBUF"):
        p = tc.tile_pool(name=name, bufs=bufs, space=space)
        return p

    with (
        pool("const") as constp,
        pool("persist") as persist,
        pool("dram", space="DRAM") as dramp,
    ):
        ones_sb = constp.tile([128, 1], F32R)
        nc.sync.dma_start(ones_sb[:], onesp[:])
        eps_sb = constp.tile([1, 1], F32)
        nc.sync.dma_start(eps_sb[:], epsp[:])
        id_sb = constp.tile([128, 128], F32R)
        nc.sync.dma_start(id_sb[:], ident[:])
        bo4_sb = constp.tile([128, NDC], F32)
        nc.sync.dma_start(bo4_sb[:], bo4[:])
        cosk_sb = constp.tile([HD, S], F32)
        nc.sync.dma_start(cosk_sb[:], cosk[:])
        sink_sb = constp.tile([HD, S], F32)
        nc.sync.dma_start(sink_sb[:], sink[:])
        cosq_sb = constp.tile([HD, K], F32)
        nc.sync.dma_start(cosq_sb[:], cosq[:])
        sinq_sb = constp.tile([HD, K], F32)
        nc.sync.dma_start(sinq_sb[:], sinq[:])
        mask_sb = constp.tile([128, max(int(moff[-1]), 1)], F32)
        nc.sync.dma_start(mask_sb[:], maskp[:])

        # Persistent intermediates
        kT_sb = persist.tile([HD, S], F32R)           # rope'd, scaled
        v_sb = [persist.tile([128, HD], F32R, tag=f"v{t}", name=f"v{t}") for t in range(NT)]
        qT_sb = [persist.tile([HD, K], F32R, tag=f"q{m}", name=f"q{m}") for m in range(QH)]
        outT_sb = [persist.tile([HD, K], F32R, tag=f"o{m}", name=f"om{m}") for m in range(QH)]

        # ---------------- Phase A1: k/v projections ----------------
        with (
            pool("wkv") as wkvp,
            pool("ha", bufs=3) as hap,
            pool("sqa", bufs=2) as sqp,
            pool("rows", bufs=2) as rowp,
            pool("pker", bufs=2, space="PSUM") as pk,
            pool("pver", bufs=2, space="PSUM") as pv,
            pool("pssq", bufs=2, space="PSUM") as pssq,
            pool("pmsq", bufs=1, space="PSUM") as pmsq,
            pool("ptr", bufs=1, space="PSUM") as ptr,
            pool("vt", bufs=2) as vtp,
        ):
            wk_sb = wkvp.tile([128, D], F32R)
            wv_sb = wkvp.tile([128, D], F32R)
            for dc in range(NDC):
                nc.sync.dma_start(wk_sb[:, dc * HD:(dc + 1) * HD],
                                  wk[dc * 128:(dc + 1) * 128, :])
                nc.sync.dma_start(wv_sb[:, dc * HD:(dc + 1) * HD],
                                  wv[dc * 128:(dc + 1) * 128, :])
            for sc in range(4):
                s0 = sc * 512
                kraw = pk.tile([128, 512], F32)
                vraw = pv.tile([128, 512], F32)
                ssq = pssq.tile([1, 512], F32)
                for dc in range(NDC):
                    ht = hap.tile([128, 512], F32R, tag="ht")
                    nc.sync.dma_start(
                        ht[:], hT[dc * 128:(dc + 1) * 128, s0:s0 + 512])
                    sq = sqp.tile([128, 512], F32R, tag="sq")
                    nc.scalar.square(sq[:], ht[:])
                    nc.tensor.matmul(ssq[:], ones_sb[:], sq[:],
                                     start=(dc == 0), stop=(dc == NDC - 1))
                    nc.tensor.matmul(kraw[:], wk_sb[:, dc * HD:(dc + 1) * HD],
                                     ht[:], start=(dc == 0),
                                     stop=(dc == NDC - 1))
                    nc.tensor.matmul(vraw[:], wv_sb[:, dc * HD:(dc + 1) * HD],
                                     ht[:], start=(dc == 0),
                                     stop=(dc == NDC - 1))
                # rs = rsqrt(ssq/D + eps)  [1,512]
                t1 = rowp.tile([1, 512], F32, tag="t1")
                nc.scalar.activation(t1[:], ssq[:],
                                     mybir.ActivationFunctionType.Sqrt,
                                     bias=eps_sb[:], scale=1.0 / D)
                rs = rowp.tile([1, 512], F32, tag="rs")
                nc.vector.reciprocal(rs[:], t1[:])
                # k-norm: msqk = sum_d kraw^2 -> comb = rs*rsqrt(rs^2*msqk/HD+eps)
                sqk = sqp.tile([128, 512], F32R, tag="sqk")
                nc.scalar.square(sqk[:], kraw[:])
                msqk = pmsq.tile([1, 512], F32)
                nc.tensor.matmul(msqk[:], ones_sb[:], sqk[:], start=True,
                                 stop=True)
                rs2 = rowp.tile([1, 512], F32, tag="rs2")
                nc.vector.tensor_mul(rs2[:], rs[:], rs[:])
                t2 = rowp.tile([1, 512], F32, tag="t2")
                nc.vector.tensor_mul(t2[:], msqk[:], rs2[:])
                t3 = rowp.tile([1, 512], F32, tag="t3")
                nc.scalar.activation(t3[:], t2[:],
                                     mybir.ActivationFunctionType.Sqrt,
                                     bias=eps_sb[:], scale=1.0 / HD)
                rsk = rowp.tile([1, 512], F32, tag="rsk")
                nc.vector.reciprocal(rsk[:], t3[:])
                comb = rowp.tile([1, 512], F32, tag="comb")
                nc.vector.tensor_mul(comb[:], rs[:], rsk[:])
                comb_bc = sqp.tile([128, 512], F32, tag="combbc")
                nc.gpsimd.partition_broadcast(comb_bc[:], comb[:])
                rs_bc = sqp.tile([128, 512], F32, tag="rsbc")
                nc.gpsimd.partition_broadcast(rs_bc[:], rs[:])
                # rope(k): kf = (kraw*cos + shuf(kraw)*sin) * comb
                kc_ = sqp.tile([128, 512], F32, tag="kc")
                nc.vector.tensor_mul(kc_[:], kraw[:], cosk_sb[:, s0:s0 + 512])
                ks = sqp.tile([128, 512], F32, tag="ks")
                nc.vector.tensor_mul(ks[0:64, :], kraw[64:128, :],
                                     sink_sb[0:64, s0:s0 + 512])
                nc.vector.tensor_mul(ks[64:128, :], kraw[0:64, :],
                                     sink_sb[64:128, s0:s0 + 512])
                kcs = sqp.tile([128, 512], F32, tag="kcs")
                nc.vector.tensor_add(kcs[:], kc_[:], ks[:])
                nc.vector.tensor_mul(kT_sb[:, s0:s0 + 512], kcs[:],
                                     comb_bc[:])
                # v: scale then transpose per 128-block
                vts = vtp.tile([128, 512], F32R, tag="vts")
                nc.vector.tensor_mul(vts[:], vraw[:], rs_bc[:])
                for j in range(4):
                    vps = ptr.tile([128, 128], F32R, tag="vps")
                    nc.tensor.transpose(vps[:], vts[:, j * 128:(j + 1) * 128],
                                        id_sb[:])
                    nc.scalar.copy(v_sb[sc * 4 + j][:], vps[:])

        # ---------------- Phase A2: q projection ----------------
        with (
            pool("wqp") as wqp,
            pool("hq", bufs=3) as hqp,
            pool("sqb", bufs=2) as sqbp,
            pool("rowq", bufs=2) as rowqp,
            pool("pqr", bufs=1, space="PSUM") as pq,
            pool("pssq2", bufs=2, space="PSUM") as pssq2,
            pool("pmsq2", bufs=2, space="PSUM") as pmsq2,
        ):
            wq_sb = wqp.tile([128, NDC * GW], F32R)
            for dc in range(NDC):
                nc.sync.dma_start(wq_sb[:, dc * GW:(dc + 1) * GW],
                                  wq[dc * 128:(dc + 1) * 128, :])
            for kc in range(2):
                k0 = kc * 512
                qraw = [pq.tile([128, 512], F32, tag=f"qraw{m}", name=f"qraw{m}")
                        for m in range(QH)]
                ssqq = pssq2.tile([1, 512], F32)
                for dc in range(NDC):
                    hqt = hqp.tile([128, 512], F32R, tag="hqt")
                    nc.sync.dma_start(
                        hqt[:], hqT[dc * 128:(dc + 1) * 128, k0:k0 + 512])
                    sq = sqbp.tile([128, 512], F32R, tag="sqq")
                    nc.scalar.square(sq[:], hqt[:])
                    nc.tensor.matmul(ssqq[:], ones_sb[:], sq[:],
                                     start=(dc == 0), stop=(dc == NDC - 1))
                    for m in range(QH):
                        nc.tensor.matmul(
                            qraw[m][:],
                            wq_sb[:, dc * GW + m * HD: dc * GW + (m + 1) * HD],
                            hqt[:], start=(dc == 0), stop=(dc == NDC - 1))
                t1 = rowqp.tile([1, 512], F32, tag="t1")
                nc.scalar.activation(t1[:], ssqq[:],
                                     mybir.ActivationFunctionType.Sqrt,
                                     bias=eps_sb[:], scale=1.0 / D)
                rsq_ln = rowqp.tile([1, 512], F32, tag="rsln")
                nc.vector.reciprocal(rsq_ln[:], t1[:])
                rs2 = rowqp.tile([1, 512], F32, tag="rs2")
                nc.vector.tensor_mul(rs2[:], rsq_ln[:], rsq_ln[:])
                for m in range(QH):
                    sqm = sqbp.tile([128, 512], F32R, tag="sqm")
                    nc.scalar.square(sqm[:], qraw[m][:])
                    msqq = pmsq2.tile([1, 512], F32, tag="msqq")
                    nc.tensor.matmul(msqq[:], ones_sb[:], sqm[:], start=True,
                                     stop=True)
                    t2 = rowqp.tile([1, 512], F32, tag="t2")
                    nc.vector.tensor_mul(t2[:], msqq[:], rs2[:])
                    t3 = rowqp.tile([1, 512], F32, tag="t3")
                    nc.scalar.activation(t3[:], t2[:],
                                         mybir.ActivationFunctionType.Sqrt,
                                         bias=eps_sb[:], scale=1.0 / HD)
                    rsq = rowqp.tile([1, 512], F32, tag="rsq")
                    nc.vector.reciprocal(rsq[:], t3[:])
                    comb = rowqp.tile([1, 512], F32, tag="comb")
                    nc.vector.tensor_mul(comb[:], rsq_ln[:], rsq[:])
                    comb_bc = sqbp.tile([128, 512], F32, tag="combbc")
                    nc.gpsimd.partition_broadcast(comb_bc[:], comb[:])
                    qc = sqbp.tile([128, 512], F32, tag="qc")
                    nc.vector.tensor_mul(qc[:], qraw[m][:],
                                         cosq_sb[:, k0:k0 + 512])
                    qs = sqbp.tile([128, 512], F32, tag="qs")
                    nc.vector.tensor_mul(qs[0:64, :], qraw[m][64:128, :],
                                         sinq_sb[0:64, k0:k0 + 512])
                    nc.vector.tensor_mul(qs[64:128, :], qraw[m][0:64, :],
                                         sinq_sb[64:128, k0:k0 + 512])
                    qcs = sqbp.tile([128, 512], F32, tag="qcs")
                    nc.vector.tensor_add(qcs[:], qc[:], qs[:])
                    nc.vector.tensor_mul(qT_sb[m][:, k0:k0 + 512], qcs[:],
                                         comb_bc[:])

        # ---------------- Phase B: attention per head ----------------
        with (
            pool("expp") as expp,
            pool("rowb", bufs=2) as rowbp,
            pool("psc", bufs=2, space="PSUM") as psc,
            pool("psum_r", bufs=1, space="PSUM") as psr,
            pool("psum_o", bufs=1, space="PSUM") as pso,
        ):
            last_t = max(tt for tt in range(NT) if klo_u[tt] < K)
            for m in range(QH):
                rsum = psr.tile([1, K], F32, tag="rsum")
                outp = pso.tile([HD, K], F32, tag="outp")
                for t in range(NT):
                    lo = klo_u[t]
                    if lo >= K:
                        continue
                    sc_ps = psc.tile([128, K], F32, tag="scps")
                    for (a, b) in _chunks(lo, K):
                        nc.tensor.matmul(
                            sc_ps[:, a:b],
                            kT_sb[:, t * 128:(t + 1) * 128],
                            qT_sb[m][:, a:b], start=True, stop=True)
                    et = expp.tile([128, K - lo], F32R, tag=f"exp{t}")
                    nc.scalar.activation(et[:], sc_ps[:, lo:],
                                         mybir.ActivationFunctionType.Exp,
                                         scale=SCALE)
                    if khi_max[t] > lo:
                        w = khi_max[t] - lo
                        nc.vector.tensor_mul(
                            et[:, 0:w], et[:, 0:w],
                            mask_sb[:, int(moff[t]):int(moff[t]) + w])
                    first = (t == 0)
                    for (a, b) in _chunks(lo, K):
                        nc.tensor.matmul(rsum[:, a:b], ones_sb[:],
                                         et[:, a - lo:b - lo],
                                         start=first, stop=(t == last_t))
                        nc.tensor.matmul(outp[:, a:b], v_sb[t][:],
                                         et[:, a - lo:b - lo],
                                         start=first, stop=(t == last_t))
                rsums = rowbp.tile([1, K], F32, tag="rsums")
                nc.scalar.copy(rsums[:], rsum[:])
                recip = rowbp.tile([1, K], F32, tag="recip")
                nc.vector.reciprocal(recip[:], rsums[:])
                recip_bc = rowbp.tile([128, K], F32, tag="recipbc")
                nc.gpsimd.partition_broadcast(recip_bc[:], recip[:])
                nc.vector.tensor_mul(outT_sb[m][:], outp[:], recip_bc[:])

        # ---------------- Phase C: o_proj + ReduceScatter ----------------
        with (
            pool("wop") as wop,
            pool("oevict", bufs=3) as oev,
            pool("poT", bufs=2, space="PSUM") as poT,
        ):
            wo_sb = [wop.tile([128, D], F32R, tag=f"wo{m}", name=f"wo{m}") for m in range(QH)]
            for m in range(QH):
                nc.sync.dma_start(wo_sb[m][:], wo[m * 128:(m + 1) * 128, :])
            o_part = dramp.tile([D, K], F32)
            o_shard = dramp.tile([D // 4, K], F32)
            for dc in range(NDC):
                ops = poT.tile([128, K], F32, tag="ops")
                for m in range(QH):
                    for (a, b) in _chunks(0, K):
                        nc.tensor.matmul(
                            ops[:, a:b],
                            wo_sb[m][:, dc * 128:(dc + 1) * 128],
                            outT_sb[m][:, a:b],
                            start=(m == 0), stop=(m == QH - 1))
                osb = oev.tile([128, K], F32, tag="osb")
                nc.scalar.activation(osb[:], ops[:],
                                     mybir.ActivationFunctionType.Identity,
                                     bias=bo4_sb[:, dc:dc + 1], scale=1.0)
                nc.sync.dma_start(o_part[dc * 128:(dc + 1) * 128, :], osb[:])
            nc.gpsimd.collective_compute(
                "ReduceScatter", mybir.AluOpType.add,
                replica_groups=[[0, 1, 2, 3], [4, 5, 6, 7]],
                ins=[o_part.opt()], outs=[o_shard.opt()])
            nc.sync.dma_start(oshard[:], o_shard[:])


def kernel(hidden_states, pos_ids, cos, sin, w_ln, w_qn, w_kn,
           Wq, Wk, Wv, Wo, bo):
    h = np.ascontiguousarray(np.asarray(hidden_states, dtype=np.float32))
    pos = np.asarray(pos_ids)
    cos0 = np.asarray(cos, dtype=np.float32)[0]          # [S, HD]
    sin0 = np.asarray(sin, dtype=np.float32)[0]
    w_ln = np.asarray(w_ln, dtype=np.float32)
    w_qn = np.asarray(w_qn, dtype=np.float32)
    w_kn = np.asarray(w_kn, dtype=np.float32)
    Wq = np.asarray(Wq, dtype=np.float32)
    Wk = np.asarray(Wk, dtype=np.float32)
    Wv = np.asarray(Wv, dtype=np.float32)
    Wo = np.asarray(Wo, dtype=np.float32)
    bo = np.asarray(bo, dtype=np.float32)

    order = np.argsort(pos, axis=1, kind="stable")
    pos_s = np.take_along_axis(pos, order, axis=1)       # sorted per batch

    klo = np.stack([np.searchsorted(pos_s[b], np.arange(NT + 1) * 128)
                    for b in range(B)])                   # [B, NT+1]
    # fp32r matmul ISA: PSUM dst offsets 8B-aligned, moving N even ->
    # round computed-range starts down to a multiple of 8 columns.
    klo_u = ((klo[:, :NT].min(axis=0) // 8) * 8).astype(int).tolist()
    khi_max = klo[:, 1:].max(axis=0).astype(int).tolist()

    key = (tuple(klo_u), tuple(khi_max))
    if key not in _BUILD_CACHE:
        _BUILD_CACHE[key] = _build(klo_u, khi_max)
    nc = _BUILD_CACHE[key]

    # host-folded weights
    Wq_f = w_ln[:, None] * Wq
    Wk_f = w_ln[:, None] * Wk
    Wv_f = w_ln[:, None] * Wv

    sgn = np.where(np.arange(HD) < 64, -1.0, 1.0).astype(np.float32)[:, None]
    wqn_sh = np.roll(w_qn, -64)[:, None]
    wkn_sh = np.roll(w_kn, -64)[:, None]
    COSK = np.ascontiguousarray(w_kn[:, None] * cos0.T)
    SINK = np.ascontiguousarray(wkn_sh * sin0.T * sgn)

    mw = [max(0, khi_max[t] - klo_u[t]) for t in range(NT)]
    moff = np.concatenate([[0], np.cumsum(mw)]).astype(int)
    MW = max(int(moff[-1]), 1)

    p_arange = np.arange(128)[:, None]
    in_maps = []
    for c in range(NCORES):
        b, g = c // 4, c % 4
        ps = pos_s[b]
        hTb = np.ascontiguousarray(h[b].T)
        hqTb = np.ascontiguousarray(h[b][ps].T)
        COSQ = np.ascontiguousarray(w_qn[:, None] * cos0[ps].T)
        SINQ = np.ascontiguousarray(wqn_sh * sin0[ps].T * sgn)
        maskp = np.zeros((128, MW), dtype=np.float32)
        for t in range(NT):
            if mw[t] == 0:
                continue
            cols = ps[klo_u[t]:klo_u[t] + mw[t]][None, :]
            maskp[:, int(moff[t]):int(moff[t]) + mw[t]] = (
                (t * 128 + p_arange) <= cols).astype(np.float32)
        in_maps.append({
            "hT": hTb,
            "hqT": hqTb,
            "wq": np.ascontiguousarray(Wq_f[:, g * GW:(g + 1) * GW]),
            "wk": np.ascontiguousarray(Wk_f[:, g * HD:(g + 1) * HD]),
            "wv": np.ascontiguousarray(Wv_f[:, g * HD:(g + 1) * HD]),
            "wo": np.ascontiguousarray(Wo[g * GW:(g + 1) * GW, :]),
            "cosq": COSQ, "sinq": SINQ, "cosk": COSK, "sink": SINK,
            "maskp": maskp,
            "bo4": np.ascontiguousarray((bo / 4.0).reshape(NDC, 128).T),
            "onesp": np.ones((128, 1), dtype=np.float32),
            "epsp": np.full((1, 1), EPS, dtype=np.float32),
            "ident": np.eye(128, dtype=np.float32),
        })

    global _LAST_IN_MAPS
    _LAST_IN_MAPS = in_maps
    res = run_bass_kernel_spmd(nc, in_maps, list(range(NCORES)))

    out = np.zeros((B, S, D), dtype=np.float32)
    for b in range(B):
        oT = np.concatenate(
            [res.results[4 * b + g]["oshard"] for g in range(4)], axis=0)
        out[b, pos_s[b], :] = oT.T
    return out


# revision 9
# speedup vs baseline: 1.1749x; 1.1749x over previous
"""Sparse-attention wrapper kernel for 8 trn2 NeuronCores.

Sharding: core c -> (b = c // 4, g = c % 4). Data-parallel over batch B=2,
tensor-parallel over the 4 KV head groups (4 q-heads / 1 kv-head each).
Per-core pipeline (big matmuls in fp32r on the PE):
  A) streamed projections: kT/vT = Wk/Wv^T @ hiddenT, qT = Wq^T @ hidden_qT,
     with RMS statistics (ln-norm + q/k-norm) computed via all-ones-matrix
     matmuls in broadcast-row form and fused as column scales; RoPE applied
     via host-precomputed cos/sin factor tensors (w_qn/w_kn folded in).
  B) per-head attention in transposed orientation: scoresT[s,k] tiles,
     exp on ACT, causal handling via host-derived column ranges + boundary
     masks, unnormalized attn@v + row-sums (ones-matmul), late 1/rowsum
     normalization with a fast approximate reciprocal.
  C) o_proj into oT[D,K] partials (+ bo/4), pipelined 4-chunk on-device
     ReduceScatter over the 4 cores of each batch; host scatters the K rows
     back into [B,S,D] zeros.
"""

import numpy as np
import ml_dtypes
import concourse.bacc as bacc
import concourse.tile as tile
from concourse import mybir
from concourse.bass_utils import run_bass_kernel_spmd

B, S, K, D, H, HKV, HD = 2, 2048, 1024, 2048, 16, 4, 128
EPS = 1e-6
SCALE = float(HD) ** -0.5
NCORES = 8
NT = S // 128          # 16 s-tiles
NDC = D // 128         # 16 d-chunks
QH = H // HKV          # 4 q-heads per core
GW = QH * HD           # 512 columns of Wq per core

F32 = mybir.dt.float32
F32R = mybir.dt.float32r
BF16 = mybir.dt.bfloat16
AFT = mybir.ActivationFunctionType

_BUILD_CACHE = {}
_LAST_IN_MAPS = None


def _chunks(lo, hi, bank=512):
    """Split [lo, hi) at multiples of `bank` (PSUM bank boundaries)."""
    out = []
    a = lo
    while a < hi:
        b = min(hi, (a // bank + 1) * bank)
        out.append((a, b))
        a = b
    return out


def _build(klo_u, khi_max):
    nc = bacc.Bacc("TRN2", target_bir_lowering=False, debug=False,
                   num_devices=NCORES)

    mw = [max(0, khi_max[t] - klo_u[t]) for t in range(NT)]
    moff = np.concatenate([[0], np.cumsum(mw)]).astype(int)
    MW = int(moff[-1])

    p = {}
    p["hT"] = nc.declare_dram_parameter("hT", [D, S], F32R, isOutput=False)
    p["hqT"] = nc.declare_dram_parameter("hqT", [D, K], F32R, isOutput=False)
    p["wq"] = nc.declare_dram_parameter("wq", [D, GW], F32R, isOutput=False)
    p["wk"] = nc.declare_dram_parameter("wk", [D, HD], F32R, isOutput=False)
    p["wv"] = nc.declare_dram_parameter("wv", [D, HD], F32R, isOutput=False)
    p["wo"] = nc.declare_dram_parameter("wo", [GW, D], F32R, isOutput=False)
    p["cosq"] = nc.declare_dram_parameter("cosq", [HD, K], F32, isOutput=False)
    p["sinq"] = nc.declare_dram_parameter("sinq", [HD, K], F32, isOutput=False)
    p["cosk"] = nc.declare_dram_parameter("cosk", [HD, S], F32, isOutput=False)
    p["sink"] = nc.declare_dram_parameter("sink", [HD, S], F32, isOutput=False)
    p["maskp"] = nc.declare_dram_parameter("maskp", [128, max(MW, 1)], F32,
                                           isOutput=False)
    p["bo4"] = nc.declare_dram_parameter("bo4", [128, NDC], F32,
                                         isOutput=False)
    p["ones128"] = nc.declare_dram_parameter("ones128", [128, 128], F32R,
                                             isOutput=False)
    p["ones128h"] = nc.declare_dram_parameter("ones128h", [128, 128], BF16,
                                              isOutput=False)
    p["epsp"] = nc.declare_dram_parameter("epsp", [128, 1], F32,
                                          isOutput=False)
    p["ident"] = nc.declare_dram_parameter("ident", [128, 128], F32R,
                                           isOutput=False)
    p["oshard"] = nc.declare_dram_parameter("oshard", [D // 4, K], F32,
                                            isOutput=True)

    with tile.TileContext(nc) as tc:
        _emit(nc, tc, p, klo_u, khi_max, moff)
    nc.finalize()
    return nc


def _emit(nc, tc, p, klo_u, khi_max, moff):
    pool = lambda name, bufs=1, space="SBUF": tc.tile_pool(
        name=name, bufs=bufs, space=space)

    with (
        pool("const") as constp,
        pool("persist") as persist,
        pool("dram", space="DRAM") as dramp,
    ):
        ones_sb = constp.tile([128, 128], F32R, name="ones_sb")
        nc.sync.dma_start(ones_sb[:], p["ones128"][:])
        onesh_sb = constp.tile([128, 128], BF16, name="onesh_sb")
        nc.sync.dma_start(onesh_sb[:], p["ones128h"][:])
        eps_sb = constp.tile([128, 1], F32, name="eps_sb")
        nc.sync.dma_start(eps_sb[:], p["epsp"][:])
        id_sb = constp.tile([128, 128], F32R, name="id_sb")
        nc.sync.dma_start(id_sb[:], p["ident"][:])
        bo4_sb = constp.tile([128, NDC], F32, name="bo4_sb")
        nc.sync.dma_start(bo4_sb[:], p["bo4"][:])
        cosk_sb = constp.tile([HD, S], F32, name="cosk_sb")
        nc.sync.dma_start(cosk_sb[:], p["cosk"][:])
        sink_sb = constp.tile([HD, S], F32, name="sink_sb")
        nc.sync.dma_start(sink_sb[:], p["sink"][:])
        cosq_sb = constp.tile([HD, K], F32, name="cosq_sb")
        nc.sync.dma_start(cosq_sb[:], p["cosq"][:])
        sinq_sb = constp.tile([HD, K], F32, name="sinq_sb")
        nc.sync.dma_start(sinq_sb[:], p["sinq"][:])
        mask_sb = constp.tile([128, max(int(moff[-1]), 1)], F32,
                              name="mask_sb")
        nc.sync.dma_start(mask_sb[:], p["maskp"][:])

        kT_sb = persist.tile([HD, S], F32R, name="kT_sb")
        v_sb = [persist.tile([128, HD], F32R, tag=f"v{t}", name=f"v{t}")
                for t in range(NT)]
        qT_sb = [persist.tile([HD, K], F32R, tag=f"q{m}", name=f"q{m}")
                 for m in range(QH)]
        outT_sb = [persist.tile([HD, K], F32R, tag=f"o{m}", name=f"om{m}")
                   for m in range(QH)]

        # ---------------- Phase A1: k/v projections ----------------
        with (
            pool("wkv") as wkvp,
            pool("ha", bufs=3) as hap,
            pool("sqa") as sqp,
            pool("rowa") as rowp,
            pool("pbig", bufs=3, space="PSUM") as pbig,
            pool("ptr", bufs=2, space="PSUM") as ptr,
            pool("vt") as vtp,
        ):
            wk_sb = wkvp.tile([128, D], F32R, name="wk_sb")
            wv_sb = wkvp.tile([128, D], F32R, name="wv_sb")
            for dc in range(NDC):
                nc.sync.dma_start(wk_sb[:, dc * HD:(dc + 1) * HD],
                                  p["wk"][dc * 128:(dc + 1) * 128, :])
                nc.sync.dma_start(wv_sb[:, dc * HD:(dc + 1) * HD],
                                  p["wv"][dc * 128:(dc + 1) * 128, :])
            sq = [sqp.tile([128, 1024], BF16, tag=f"sq{dc}", name=f"sq{dc}")
                  for dc in range(NDC)]
            for sch in range(2):
                s0 = sch * 1024
                kraw = pbig.tile([128, 1024], F32, tag="big", name="kraw")
                vraw = pbig.tile([128, 1024], F32, tag="big", name="vraw")
                ssq = pbig.tile([128, 1024], F32, tag="big", name="ssq")
                for dc in range(NDC):
                    ht = hap.tile([128, 1024], F32R, tag="ht", name="ht")
                    nc.sync.dma_start(
                        ht[:], p["hT"][dc * 128:(dc + 1) * 128, s0:s0 + 1024])
                    nc.gpsimd.tensor_mul(sq[dc][:], ht[:], ht[:])
                    for (a, b) in ((0, 512), (512, 1024)):
                        nc.tensor.matmul(kraw[:, a:b],
                                         wk_sb[:, dc * HD:(dc + 1) * HD],
                                         ht[:, a:b], start=(dc == 0),
                                         stop=(dc == NDC - 1))
                    for (a, b) in ((0, 512), (512, 1024)):
                        nc.tensor.matmul(vraw[:, a:b],
                                         wv_sb[:, dc * HD:(dc + 1) * HD],
                                         ht[:, a:b], start=(dc == 0),
                                         stop=(dc == NDC - 1))
                for (a, b) in ((0, 512), (512, 1024)):
                    for dc in range(NDC):
                        nc.tensor.matmul(ssq[:, a:b], onesh_sb[:],
                                         sq[dc][:, a:b], start=(dc == 0),
                                         stop=(dc == NDC - 1))
                # rs = rsqrt(ssq/D + eps), broadcast-row form [128, 1024]
                t1 = rowp.tile([128, 1024], F32, tag="t1", name="t1")
                nc.scalar.activation(t1[:], ssq[:], AFT.Sqrt,
                                     bias=eps_sb[:], scale=1.0 / D)
                rs = rowp.tile([128, 1024], F32, tag="rs", name="rs")
                nc.vector.reciprocal_approx_fast(rs[:], t1[:])
                # v: scale by rs then transpose per 128 block
                vts = vtp.tile([128, 1024], F32R, tag="vts", name="vts")
                nc.vector.tensor_mul(vts[:], vraw[:], rs[:])
                for j in range(8):
                    vps = ptr.tile([128, 128], F32R, tag="vps", name="vps")
                    nc.tensor.transpose(vps[:], vts[:, j * 128:(j + 1) * 128],
                                        id_sb[:])
                    nc.scalar.copy(v_sb[sch * 8 + j][:], vps[:])
                # k-norm stats
                sqk = sqp.tile([128, 1024], BF16, tag="sqk", name="sqk")
                nc.scalar.square(sqk[:], kraw[:])
                msqk = pbig.tile([128, 1024], F32, tag="big", name="msqk")
                for (a, b) in ((0, 512), (512, 1024)):
                    nc.tensor.matmul(msqk[:, a:b], onesh_sb[:], sqk[:, a:b],
                                     start=True, stop=True)
                rs2 = rowp.tile([128, 1024], F32, tag="rs2", name="rs2")
                nc.vector.tensor_mul(rs2[:], rs[:], rs[:])
                t2 = rowp.tile([128, 1024], F32, tag="t2", name="t2")
                nc.vector.tensor_mul(t2[:], msqk[:], rs2[:])
                t3 = rowp.tile([128, 1024], F32, tag="t3", name="t3")
                nc.scalar.activation(t3[:], t2[:], AFT.Sqrt,
                                     bias=eps_sb[:], scale=1.0 / HD)
                rsk = rowp.tile([128, 1024], F32, tag="rsk", name="rsk")
                nc.vector.reciprocal_approx_fast(rsk[:], t3[:])
                comb = rowp.tile([128, 1024], F32, tag="comb", name="comb")
                nc.vector.tensor_mul(comb[:], rs[:], rsk[:])
                # rope(k) * comb
                kc_ = vtp.tile([128, 1024], F32, tag="kc", name="kc_")
                nc.vector.tensor_mul(kc_[:], kraw[:], cosk_sb[:, s0:s0 + 1024])
                ks = vtp.tile([128, 1024], F32, tag="ks", name="ks")
                nc.vector.tensor_mul(ks[0:64, :], kraw[64:128, :],
                                     sink_sb[0:64, s0:s0 + 1024])
                nc.vector.tensor_mul(ks[64:128, :], kraw[0:64, :],
                                     sink_sb[64:128, s0:s0 + 1024])
                kcs = vtp.tile([128, 1024], F32, tag="kcs", name="kcs")
                nc.vector.tensor_add(kcs[:], kc_[:], ks[:])
                nc.vector.tensor_mul(kT_sb[:, s0:s0 + 1024], kcs[:], comb[:])

        # ---------------- Phase A2: q projection ----------------
        with (
            pool("wqp") as wqp,
            pool("hq", bufs=3) as hqp,
            pool("sqb") as sqbp,
            pool("rowq") as rowqp,
            pool("pq", bufs=1, space="PSUM") as pq,
            pool("pqs", bufs=2, space="PSUM") as pqs,
        ):
            wq_sb = wqp.tile([128, NDC * GW], F32R, name="wq_sb")
            for dc in range(NDC):
                nc.sync.dma_start(wq_sb[:, dc * GW:(dc + 1) * GW],
                                  p["wq"][dc * 128:(dc + 1) * 128, :])
            sqq = [sqbp.tile([128, 512], BF16, tag=f"sqq{dc}", name=f"sqq{dc}")
                   for dc in range(NDC)]
            for kc in range(2):
                k0 = kc * 512
                qraw = [pq.tile([128, 512], F32, tag=f"qraw{m}",
                                name=f"qraw{m}") for m in range(QH)]
                ssqq = pqs.tile([128, 512], F32, tag="ssqq", name="ssqq")
                for dc in range(NDC):
                    hqt = hqp.tile([128, 512], F32R, tag="hqt", name="hqt")
                    nc.sync.dma_start(
                        hqt[:], p["hqT"][dc * 128:(dc + 1) * 128, k0:k0 + 512])
                    nc.gpsimd.tensor_mul(sqq[dc][:], hqt[:], hqt[:])
                    for m in range(QH):
                        nc.tensor.matmul(
                            qraw[m][:],
                            wq_sb[:, dc * GW + m * HD: dc * GW + (m + 1) * HD],
                            hqt[:], start=(dc == 0), stop=(dc == NDC - 1))
                for dc in range(NDC):
                    nc.tensor.matmul(ssqq[:], onesh_sb[:], sqq[dc][:],
                                     start=(dc == 0), stop=(dc == NDC - 1))
                t1 = rowqp.tile([128, 512], F32, tag="t1", name="t1q")
                nc.scalar.activation(t1[:], ssqq[:], AFT.Sqrt,
                                     bias=eps_sb[:], scale=1.0 / D)
                rsln = rowqp.tile([128, 512], F32, tag="rsln", name="rsln")
                nc.vector.reciprocal_approx_fast(rsln[:], t1[:])
                rs2 = rowqp.tile([128, 512], F32, tag="rs2", name="rs2q")
                nc.vector.tensor_mul(rs2[:], rsln[:], rsln[:])
                for m in range(QH):
                    sqm = sqbp.tile([128, 512], BF16, tag="sqm", name="sqm")
                    nc.scalar.square(sqm[:], qraw[m][:])
                    msqq = pqs.tile([128, 512], F32, tag="msqq", name="msqq")
                    nc.tensor.matmul(msqq[:], onesh_sb[:], sqm[:], start=True,
                                     stop=True)
                    t2 = rowqp.tile([128, 512], F32, tag="t2", name="t2q")
                    nc.vector.tensor_mul(t2[:], msqq[:], rs2[:])
                    t3 = rowqp.tile([128, 512], F32, tag="t3", name="t3q")
                    nc.scalar.activation(t3[:], t2[:], AFT.Sqrt,
                                         bias=eps_sb[:], scale=1.0 / HD)
                    rsq = rowqp.tile([128, 512], F32, tag="rsq", name="rsq")
                    nc.vector.reciprocal_approx_fast(rsq[:], t3[:])
                    comb = rowqp.tile([128, 512], F32, tag="comb", name="combq")
                    nc.vector.tensor_mul(comb[:], rsln[:], rsq[:])
                    qc = rowqp.tile([128, 512], F32, tag="qc", name="qc")
                    nc.vector.tensor_mul(qc[:], qraw[m][:],
                                         cosq_sb[:, k0:k0 + 512])
                    qs = rowqp.tile([128, 512], F32, tag="qs", name="qs")
                    nc.vector.tensor_mul(qs[0:64, :], qraw[m][64:128, :],
                                         sinq_sb[0:64, k0:k0 + 512])
                    nc.vector.tensor_mul(qs[64:128, :], qraw[m][0:64, :],
                                         sinq_sb[64:128, k0:k0 + 512])
                    qcs = rowqp.tile([128, 512], F32, tag="qcs", name="qcs")
                    nc.vector.tensor_add(qcs[:], qc[:], qs[:])
                    nc.vector.tensor_mul(qT_sb[m][:, k0:k0 + 512], qcs[:],
                                         comb[:])

        # ---------------- Phase B: attention per head ----------------
        with (
            pool("expp") as expp,
            pool("rowb", bufs=2) as rowbp,
            pool("psc", bufs=2, space="PSUM") as psc,
            pool("pro", bufs=1, space="PSUM") as pro,
        ):
            last_t = max(tt for tt in range(NT) if klo_u[tt] < K)
            for m in range(QH):
                rsum = pro.tile([128, K], F32, tag="rsum", name="rsum")
                outp = pro.tile([HD, K], F32, tag="outp", name="outp")
                ets = {}
                for t in range(NT):
                    lo = klo_u[t]
                    if lo >= K:
                        continue
                    sc_ps = psc.tile([128, K], F32, tag="scps", name="scps")
                    for (a, b) in _chunks(lo, K):
                        nc.tensor.matmul(
                            sc_ps[:, a:b], kT_sb[:, t * 128:(t + 1) * 128],
                            qT_sb[m][:, a:b], start=True, stop=True)
                    et = expp.tile([128, K - lo], F32R, tag=f"exp{t}",
                                   name=f"exp{t}")
                    ets[t] = et
                    nc.scalar.activation(et[:], sc_ps[:, lo:], AFT.Exp,
                                         scale=SCALE)
                    if khi_max[t] > lo:
                        w = khi_max[t] - lo
                        nc.vector.tensor_mul(
                            et[:, 0:w], et[:, 0:w],
                            mask_sb[:, int(moff[t]):int(moff[t]) + w])
                # row sums (ones128 loaded once)
                for t in range(NT):
                    lo = klo_u[t]
                    if lo >= K:
                        continue
                    for (a, b) in _chunks(lo, K):
                        nc.tensor.matmul(rsum[:, a:b], ones_sb[:],
                                         ets[t][:, a - lo:b - lo],
                                         start=(t == 0), stop=(t == last_t))
                # attn @ v
                for t in range(NT):
                    lo = klo_u[t]
                    if lo >= K:
                        continue
                    for (a, b) in _chunks(lo, K):
                        nc.tensor.matmul(outp[:, a:b], v_sb[t][:],
                                         ets[t][:, a - lo:b - lo],
                                         start=(t == 0), stop=(t == last_t))
                recip = rowbp.tile([128, K], F32, tag="recip", name="recip")
                nc.vector.reciprocal_approx_fast(recip[:], rsum[:])
                nc.vector.tensor_mul(outT_sb[m][:], outp[:], recip[:])

        # ---------------- Phase C: o_proj + chunked ReduceScatter --------
        with (
            pool("wop") as wop,
            pool("oevict", bufs=3) as oev,
            pool("poT", bufs=2, space="PSUM") as poT,
        ):
            wo_sb = [wop.tile([128, D], F32R, tag=f"wo{m}", name=f"wo{m}")
                     for m in range(QH)]
            for m in range(QH):
                nc.sync.dma_start(wo_sb[m][:],
                                  p["wo"][m * 128:(m + 1) * 128, :])
            o_part = dramp.tile([D, K], F32, name="o_part")
            o_shard = [dramp.tile([128, K], F32, tag=f"osh{qd}",
                                  name=f"osh{qd}") for qd in range(4)]
            for qd in range(4):
                for dci in range(4):
                    dc = qd * 4 + dci
                    ops = poT.tile([128, K], F32, tag="ops", name="ops")
                    for m in range(QH):
                        for (a, b) in _chunks(0, K):
                            nc.tensor.matmul(
                                ops[:, a:b],
                                wo_sb[m][:, dc * 128:(dc + 1) * 128],
                                outT_sb[m][:, a:b],
                                start=(m == 0), stop=(m == QH - 1))
                    osb = oev.tile([128, K], F32, tag="osb", name="osb")
                    nc.scalar.activation(osb[:], ops[:], AFT.Identity,
                                         bias=bo4_sb[:, dc:dc + 1], scale=1.0)
                    nc.sync.dma_start(o_part[dc * 128:(dc + 1) * 128, :],
                                      osb[:])
                nc.gpsimd.collective_compute(
                    "ReduceScatter", mybir.AluOpType.add,
                    replica_groups=[[0, 1, 2, 3], [4, 5, 6, 7]],
                    ins=[o_part[qd * 512:(qd + 1) * 512, :]],
                    outs=[o_shard[qd].opt()])
                nc.sync.dma_start(
                    p["oshard"][qd * 128:(qd + 1) * 128, :], o_shard[qd][:])


def kernel(hidden_states, pos_ids, cos, sin, w_ln, w_qn, w_kn,
           Wq, Wk, Wv, Wo, bo):
    h = np.ascontiguousarray(np.asarray(hidden_states, dtype=np.float32))
    pos = np.asarray(pos_ids)
    cos0 = np.asarray(cos, dtype=np.float32)[0]          # [S, HD]
    sin0 = np.asarray(sin, dtype=np.float32)[0]
    w_ln = np.asarray(w_ln, dtype=np.float32)
    w_qn = np.asarray(w_qn, dtype=np.float32)
    w_kn = np.asarray(w_kn, dtype=np.float32)
    Wq = np.asarray(Wq, dtype=np.float32)
    Wk = np.asarray(Wk, dtype=np.float32)
    Wv = np.asarray(Wv, dtype=np.float32)
    Wo = np.asarray(Wo, dtype=np.float32)
    bo = np.asarray(bo, dtype=np.float32)

    order = np.argsort(pos, axis=1, kind="stable")
    pos_s = np.take_along_axis(pos, order, axis=1)       # sorted per batch

    klo = np.stack([np.searchsorted(pos_s[b], np.arange(NT + 1) * 128)
                    for b in range(B)])                   # [B, NT+1]
    # fp32r matmul ISA: PSUM dst offsets 8B-aligned, moving N even ->
    # round computed-range starts down to a multiple of 8 columns.
    klo_u = ((klo[:, :NT].min(axis=0) // 8) * 8).astype(int).tolist()
    khi_max = klo[:, 1:].max(axis=0).astype(int).tolist()

    key = (tuple(klo_u), tuple(khi_max))
    if key not in _BUILD_CACHE:
        _BUILD_CACHE[key] = _build(klo_u, khi_max)
    nc = _BUILD_CACHE[key]

    Wq_f = w_ln[:, None] * Wq
    Wk_f = w_ln[:, None] * Wk
    Wv_f = w_ln[:, None] * Wv

    sgn = np.where(np.arange(HD) < 64, -1.0, 1.0).astype(np.float32)[:, None]
    wqn_sh = np.roll(w_qn, -64)[:, None]
    wkn_sh = np.roll(w_kn, -64)[:, None]
    COSK = np.ascontiguousarray(w_kn[:, None] * cos0.T)
    SINK = np.ascontiguousarray(wkn_sh * sin0.T * sgn)

    mw = [max(0, khi_max[t] - klo_u[t]) for t in range(NT)]
    moff = np.concatenate([[0], np.cumsum(mw)]).astype(int)
    MW = max(int(moff[-1]), 1)

    p_arange = np.arange(128)[:, None]
    in_maps = []
    for c in range(NCORES):
        b, g = c // 4, c % 4
        ps = pos_s[b]
        hTb = np.ascontiguousarray(h[b].T)
        hqTb = np.ascontiguousarray(h[b][ps].T)
        COSQ = np.ascontiguousarray(w_qn[:, None] * cos0[ps].T)
        SINQ = np.ascontiguousarray(wqn_sh * sin0[ps].T * sgn)
        maskp = np.zeros((128, MW), dtype=np.float32)
        for t in range(NT):
            if mw[t] == 0:
                continue
            cols = ps[klo_u[t]:klo_u[t] + mw[t]][None, :]
            maskp[:, int(moff[t]):int(moff[t]) + mw[t]] = (
                (t * 128 + p_arange) <= cols).astype(np.float32)
        in_maps.append({
            "hT": hTb,
            "hqT": hqTb,
            "wq": np.ascontiguousarray(Wq_f[:, g * GW:(g + 1) * GW]),
            "wk": np.ascontiguousarray(Wk_f[:, g * HD:(g + 1) * HD]),
            "wv": np.ascontiguousarray(Wv_f[:, g * HD:(g + 1) * HD]),
            "wo": np.ascontiguousarray(Wo[g * GW:(g + 1) * GW, :]),
            "cosq": COSQ, "sinq": SINQ, "cosk": COSK, "sink": SINK,
            "maskp": maskp,
            "bo4": np.ascontiguousarray((bo / 4.0).reshape(NDC, 128).T),
            "ones128": np.ones((128, 128), dtype=np.float32),
            "ones128h": np.ones((128, 128), dtype=ml_dtypes.bfloat16),
            "epsp": np.full((128, 1), EPS, dtype=np.float32),
            "ident": np.eye(128, dtype=np.float32),
        })

    global _LAST_IN_MAPS
    _LAST_IN_MAPS = in_maps
    res = run_bass_kernel_spmd(nc, in_maps, list(range(NCORES)))

    out = np.zeros((B, S, D), dtype=np.float32)
    for b in range(B):
        oT = np.concatenate(
            [res.results[4 * b + g]["oshard"] for g in range(4)], axis=0)
        # oshard rows on core (b,g): [qd*512 + g*128, qd*512 + (g+1)*128)
        oTfull = np.empty((D, K), dtype=np.float32)
        for g in range(4):
            sh = res.results[4 * b + g]["oshard"]
            for qd in range(4):
                oTfull[qd * 512 + g * 128: qd * 512 + (g + 1) * 128] = \
                    sh[qd * 128:(qd + 1) * 128]
        out[b, pos_s[b], :] = oTfull.T
    return out
